# revision 17
# baseline (speedup 1.0000x reference)
"""MoE adapter kernel for 8 Trainium2 NeuronCores — v6.

v5 structure (slot-based SPMD, host routing/combine) with per-slot
mixed-precision K-splits: each slot has a level (KF1, KF2) = number of
fp8-DoubleRow k-tile pairs in the W1 / W2 contraction; the rest of the
contraction runs fp16.  Per-token error budget is allocated across the
token's two experts by a savings-per-error waterfill (g* per pair), so
top-1 experts also get partial fp8 instead of the v5 binary w<=0.48
rule.  Both W1 and W2 (fp8 and fp16 halves alike) are pre-scaled by 64
so mixed-dtype accumulation shares one PSUM scale; y is emitted fp16
(x64) and rescaled on host.

Cost model (ns, fitted from HW traces): fp16 MM = s/2.4+2.5 per k-tile;
fp8 DR pair = max(213.3, 0.4708*s); DMA = bytes/0.32GB/ms; 5us/slot.
"""

import os
import numpy as np
import ml_dtypes

B = 8192
IN_DIM = 5120
HID = 4096
OUT_DIM = 2048
E = 8
NCORES = 8
KT1 = IN_DIM // 128      # 40 k-tiles for W1
HT = HID // 128          # 32 h-tiles
KT2 = HID // 128         # 32 k-tiles for W2
OT = OUT_DIM // 128      # 16 o-tiles

W_SCALE = 64.0
C_BUDGET = 0.505         # per-element error cap (in w*sqrt(g) units)

# savings per unit f1 (W1 fp8 fraction) / f2 (W2 fp8 fraction), relative
# to the fp16 cost of one token-expert pair
_S1U = 0.311
_S2U = 0.124

# slot levels: (KF1 DR pairs of 20, KF2 DR pairs of 16) -> g = KF1/20+KF2/16
LEVELS = [(0, 0), (5, 0), (10, 0), (15, 0), (20, 0), (20, 8), (20, 16)]
LVL_G = [kf1 / 20.0 + kf2 / 16.0 for (kf1, kf2) in LEVELS]

LAST_RESULT = None

import random as _random

_MAXS = 512
_MINS = 320
_SLOT_PEN = 5000.0
_DMA_GBPNS = 320.0       # bytes per ns (~320 GB/s effective per core)


def _mm16(s):
    return s / 2.4 + 2.5


def _mm8(s):
    return max(213.3, 0.4708 * s)


def _cost_slot(lvl, s):
    kf1, kf2 = LEVELS[lvl]
    comp1 = HT * (kf1 * _mm8(s) + (KT1 - 2 * kf1) * _mm16(s))
    comp2 = OT * (kf2 * _mm8(s) + (KT2 - 2 * kf2) * _mm16(s))
    w1by = 128 * 128 * (2 * kf1 + (KT1 - 2 * kf1) * 2) * HT
    w2by = 128 * 128 * (2 * kf2 + (KT2 - 2 * kf2) * 2) * OT
    xby = 128 * s * (2 * kf1 + (KT1 - 2 * kf1) * 2)
    yby = 128 * s * 2 * OT
    # per-phase roofline: W1 phase streams w1+x, W2 phase streams w2+y
    ph1 = max(comp1, (w1by + xby) / _DMA_GBPNS)
    ph2 = max(comp2, (w2by + yby) / _DMA_GBPNS)
    return ph1 + ph2 + _SLOT_PEN


def _slot_order(slots):
    """Placement order: descending g (strictest eligibility first)."""
    return sorted(range(len(slots)), key=lambda i: (-LVL_G[slots[i][0]],
                                                    -slots[i][1]))


def _coverage_all(slots, n, elig_cnt):
    """Greedy nested coverage.  elig_cnt[e][lvl] = #pairs of e with
    g* >= LVL_G[lvl].  Returns per-expert covered counts."""
    order = _slot_order(slots)
    covered = [0] * E
    for e in range(E):
        taken = 0
        for i in order:
            lvl, s = slots[i]
            cap = n[e][i] * s
            avail = elig_cnt[e][lvl] - taken
            if avail > 0 and cap > 0:
                taken += min(cap, avail)
        covered[e] = taken
    return covered


def _makespan(slots, n, counts, elig_cnt):
    cost = sum(_cost_slot(lvl, s) for (lvl, s) in slots)
    pen = 0.0
    covered = _coverage_all(slots, n, elig_cnt)
    for e in range(E):
        deficit = counts[e] - covered[e]
        if deficit > 0:
            pen += 50000.0 + deficit * 7000.0
    for i in range(len(slots)):
        used = sum(n[e][i] for e in range(E))
        if used > NCORES:
            pen += (used - NCORES) * 3e6
    return cost + pen


def _greedy0(counts):
    """All-fp16 fallback construction (level 0 slots of size 448)."""
    slots = []
    n = [[] for _ in range(E)]
    need = list(counts)
    while any(x > 0 for x in need):
        slots.append((0, 448))
        for e in range(E):
            n[e].append(0)
        copies = NCORES
        for e in sorted(range(E), key=lambda e: -need[e]):
            while need[e] > 0 and copies > 0:
                n[e][-1] += 1
                need[e] -= 448
                copies -= 1
    return slots, n


def _seed_solution(counts, elig_cnt):
    """Heuristic start: per expert, walk pairs desc g*, make 512 slots at
    the floored level of each chunk's min g*."""
    slots = []
    n = [[] for _ in range(E)]
    # aggregate: for each level desc, capacity needed
    for lvl in range(len(LEVELS) - 1, -1, -1):
        need = []
        for e in range(E):
            hi = elig_cnt[e][lvl]
            lo = elig_cnt[e][lvl + 1] if lvl + 1 < len(LEVELS) else 0
            need.append(hi - lo)
        while any(x > 0 for x in need):
            slots.append((lvl, 512))
            for e in range(E):
                n[e].append(0)
            copies = NCORES
            for e in sorted(range(E), key=lambda e: -need[e]):
                while need[e] > 0 and copies > 0:
                    n[e][-1] += 1
                    need[e] -= 512
                    copies -= 1
    return slots, n


def _seed_v5ish(counts, elig_cnt):
    """v5-style: level-4 (g=1) slots covering eligible pairs, rest fp16."""
    slots = []
    n = [[] for _ in range(E)]
    lvl4 = LVL_G.index(1.0)
    need = [elig_cnt[e][lvl4] for e in range(E)]
    while sum(need) > 2048:
        slots.append((lvl4, 464))
        for e in range(E):
            n[e].append(0)
        copies = NCORES
        for e in sorted(range(E), key=lambda e: -need[e]):
            while need[e] > 0 and copies > 0:
                n[e][-1] += 1
                need[e] -= 464
                copies -= 1
    placed = [min(elig_cnt[e][lvl4] - max(need[e], 0), elig_cnt[e][lvl4])
              for e in range(E)]
    need = [counts[e] - placed[e] for e in range(E)]
    while any(x > 0 for x in need):
        slots.append((0, 448))
        for e in range(E):
            n[e].append(0)
        copies = NCORES
        for e in sorted(range(E), key=lambda e: -need[e]):
            while need[e] > 0 and copies > 0:
                n[e][-1] += 1
                need[e] -= 448
                copies -= 1
    return slots, n


def _solve_once(counts, elig_cnt, iters, seed, init=None):
    rng = _random.Random(seed)
    if init is not None:
        slots = [s for s in init[0]]
        n = [r[:] for r in init[1]]
    else:
        slots, n = _seed_solution(counts, elig_cnt)
    cur = _makespan(slots, n, counts, elig_cnt)
    best = (cur, [s for s in slots], [r[:] for r in n])
    nlvl = len(LEVELS)
    for it in range(iters):
        T = max(0.02, 1.0 * (1 - it / iters))
        op = rng.random()
        slots2 = [s for s in slots]
        n2 = [r[:] for r in n]
        if op < 0.25 and slots2:
            i = rng.randrange(len(slots2))
            lvl, s = slots2[i]
            s2 = min(_MAXS, max(_MINS,
                                s + rng.choice([-64, -32, -16, 16, 32, 64])))
            slots2[i] = (lvl, s2)
        elif op < 0.45 and slots2:
            i = rng.randrange(len(slots2))
            lvl, s = slots2[i]
            lvl2 = min(nlvl - 1, max(0, lvl + rng.choice([-2, -1, 1, 2])))
            slots2[i] = (lvl2, s)
        elif op < 0.60 and slots2:
            i = rng.randrange(len(slots2))
            e1 = rng.randrange(E)
            if n2[e1][i] > 0:
                n2[e1][i] -= 1
                if rng.random() < 0.8:
                    n2[rng.randrange(E)][i] += 1
        elif op < 0.72 and slots2:
            n2[rng.randrange(E)][rng.randrange(len(slots2))] += 1
        elif op < 0.85 and len(slots2) > 1:
            i = rng.randrange(len(slots2))
            del slots2[i]
            for e in range(E):
                del n2[e][i]
        else:
            lvl = rng.randrange(nlvl)
            slots2.append((lvl, rng.choice(range(_MINS, _MAXS + 1, 16))))
            for e in range(E):
                n2[e].append(0)
            i = len(slots2) - 1
            copies = NCORES
            for e in sorted(range(E), key=lambda e: -counts[e])[:NCORES]:
                if copies:
                    n2[e][i] += 1
                    copies -= 1
        c2 = _makespan(slots2, n2, counts, elig_cnt)
        if c2 <= cur or rng.random() < pow(2.718, -(c2 - cur) / (T * 30000)):
            slots, n, cur = slots2, n2, c2
            if cur < best[0]:
                best = (cur, [s for s in slots], [r[:] for r in n])
    return best


def _used_lengths(slots, n, elig_cnt):
    """Mirror placement: per slot, max copy fill (for size shrinking)."""
    order = _slot_order(slots)
    used = [0] * len(slots)
    ptr = [0] * E
    for i in order:
        lvl, sz = slots[i]
        for e in range(E):
            for _ in range(n[e][i]):
                avail = elig_cnt[e][lvl] - ptr[e]
                take = max(min(sz, avail), 0)
                ptr[e] += take
                used[i] = max(used[i], take)
    return used


def _shrink(slots, n, counts, elig_cnt):
    """Shrink slot sizes to the max actually-used copy length."""
    for _ in range(4):
        used = _used_lengths(slots, n, elig_cnt)
        new = []
        changed = False
        for i, (lvl, sz) in enumerate(slots):
            s2 = min(sz, max(_MINS, -(-used[i] // 16) * 16))
            if s2 != sz:
                changed = True
            new.append((lvl, s2))
        cov = _coverage_all(new, n, elig_cnt)
        if all(cov[e] >= counts[e] for e in range(E)):
            slots = new
        else:
            break
        if not changed:
            break
    return slots


def _solve(counts, elig_cnt, restarts=10, iters=200000):
    best = None
    seeds = [None] * restarts
    seeds[0] = _seed_v5ish(counts, elig_cnt)
    if restarts > 1:
        seeds[1] = _seed_v5ish(counts, elig_cnt)
    for r in range(restarts):
        c, slots, n = _solve_once(counts, elig_cnt, iters, seed=r,
                                  init=seeds[r])
        slots = _shrink(slots, n, counts, elig_cnt)
        c = _makespan(slots, n, counts, elig_cnt)
        if best is None or c < best[0]:
            best = (c, slots, n)
    c, slots, n = best
    keep = [i for i in range(len(slots)) if any(n[e][i] for e in range(E))]
    slots = [slots[i] for i in keep]
    n = [[r[i] for i in keep] for r in n]

    # deterministic deficit repair: lower levels / grow / add fp16 slots
    for _ in range(128):
        covered = _coverage_all(slots, n, elig_cnt)
        bad = [e for e in range(E) if covered[e] < counts[e]]
        if not bad:
            break
        e = bad[0]
        fixed = False
        # 1. grow a slot this expert uses
        for i in range(len(slots)):
            lvl, s = slots[i]
            if n[e][i] and s + 16 <= _MAXS:
                slots[i] = (lvl, s + 16)
                fixed = True
                break
        if not fixed:
            # 2. lower the level of a slot this expert uses
            for i in range(len(slots)):
                lvl, s = slots[i]
                if n[e][i] and lvl > 0:
                    slots[i] = (lvl - 1, s)
                    fixed = True
                    break
        if not fixed:
            # 3. add a copy in an existing non-full slot (level 0 pref)
            for i in sorted(range(len(slots)),
                            key=lambda i: LVL_G[slots[i][0]]):
                if sum(n[x][i] for x in range(E)) < NCORES:
                    n[e][i] += 1
                    fixed = True
                    break
        if not fixed:
            slots.append((0, 448))
            for x in range(E):
                n[x].append(1 if x == e else 0)
    covered = _coverage_all(slots, n, elig_cnt)
    if any(covered[e] < counts[e] for e in range(E)):
        slots, n = _greedy0(counts)
    slots = _shrink(slots, n, counts, elig_cnt)
    return slots, n


def _route(X, gW1, gb1, gW2, gb2):
    g = np.maximum(X.astype(np.float64) @ gW1.astype(np.float64)
                   + gb1.astype(np.float64), 0.0)
    logits = g @ gW2.astype(np.float64) + gb2.astype(np.float64)
    top2 = np.argpartition(-logits, 1, axis=1)[:, :2]
    l2 = np.take_along_axis(logits, top2, axis=1)
    ew = np.exp(l2 - l2.max(axis=1, keepdims=True))
    wts = ew / ew.sum(axis=1, keepdims=True)
    return top2, wts.astype(np.float32)


def _waterfill(wts):
    """Per-token optimal error-budget split.  wts [B,2] combine weights.
    Returns g* [B,2]: total fp8 fraction units (f1+f2, in [0,2]) each
    pair can take s.t. sum_i w_i^2 g_i <= C^2, maximizing DR savings."""
    w = wts.astype(np.float64)
    n = w.shape[0]
    budget = np.full(n, C_BUDGET * C_BUDGET)
    # items per token: (expert0 W1), (e0 W2), (e1 W1), (e1 W2)
    costs = np.stack([w[:, 0] ** 2, w[:, 0] ** 2,
                      w[:, 1] ** 2, w[:, 1] ** 2], axis=1)
    sav = np.array([_S1U, _S2U, _S1U, _S2U])
    ratio = sav[None, :] / costs
    order = np.argsort(-ratio, axis=1)
    f = np.zeros((n, 4))
    for j in range(4):
        idx = order[:, j]
        c = np.take_along_axis(costs, idx[:, None], axis=1)[:, 0]
        take = np.minimum(1.0, budget / c)
        np.put_along_axis(f, idx[:, None], take[:, None], axis=1)
        budget -= take * c
    return np.stack([f[:, 0] + f[:, 1], f[:, 2] + f[:, 3]], axis=1)


_NC = {}


def _build_bass(slot_key):
    import concourse.mybir as mybir
    import concourse.tile as tile
    from concourse import bacc
    from concourse.bass import ts

    f16 = mybir.dt.float16
    f32 = mybir.dt.float32
    f8 = mybir.dt.float8e4
    DR = mybir.MatmulPerfMode.DoubleRow
    relu = mybir.ActivationFunctionType.Relu

    nc = bacc.Bacc("TRN2", target_bir_lowering=False, debug=False,
                   num_devices=NCORES)

    slots = list(slot_key)
    dram = []
    for si, (lvl, sz) in enumerate(slots):
        kf1, kf2 = LEVELS[lvl]
        k8 = 2 * kf1              # fp8 k-tiles in W1
        k16 = KT1 - k8            # fp16 k-tiles in W1
        h8 = 2 * kf2              # fp8 k-tiles (h-tiles) in W2
        h16 = KT2 - h8
        d = {}
        if k8:
            d['xt8'] = nc.dram_tensor(f"xt8_{si}", [128, k8, sz], f8,
                                      kind="ExternalInput")
            d['w18'] = nc.dram_tensor(f"w18_{si}", [HT, 128, k8, 128], f8,
                                      kind="ExternalInput")
        if k16:
            d['xt16'] = nc.dram_tensor(f"xt16_{si}", [128, k16 * sz], f16,
                                       kind="ExternalInput")
            d['w116'] = nc.dram_tensor(f"w116_{si}", [HT, 128, k16 * 128],
                                       f16, kind="ExternalInput")
        if h8:
            d['w28'] = nc.dram_tensor(f"w28_{si}", [OT, 128, h8, 128], f8,
                                      kind="ExternalInput")
        if h16:
            d['w216'] = nc.dram_tensor(f"w216_{si}", [OT, 128, h16 * 128],
                                       f16, kind="ExternalInput")
        d['b1'] = nc.dram_tensor(f"b1_{si}", [128, HT], f32,
                                 kind="ExternalInput")
        d['yt'] = nc.dram_tensor(f"yt_{si}", [OT, 128, sz], f16,
                                 kind="ExternalOutput")
        dram.append(d)

    with tile.TileContext(nc) as tc:
        with (
            tc.tile_pool(name="xt", bufs=2) as xt_pool,
            tc.tile_pool(name="w1", bufs=3) as w1_pool,
            tc.tile_pool(name="w2", bufs=2) as w2_pool,
            tc.tile_pool(name="h", bufs=1) as h_pool,
            tc.tile_pool(name="b", bufs=2) as b_pool,
            tc.tile_pool(name="y", bufs=4) as y_pool,
            tc.tile_pool(name="ps1", bufs=2, space="PSUM") as ps1_pool,
            tc.tile_pool(name="ps2", bufs=2, space="PSUM") as ps2_pool,
        ):
            for si, (lvl, sz) in enumerate(slots):
                kf1, kf2 = LEVELS[lvl]
                k8 = 2 * kf1
                k16 = KT1 - k8
                h8n = 2 * kf2
                h16n = KT2 - h8n
                d = dram[si]

                b1t = b_pool.tile([128, HT], f32, tag="b1")
                nc.sync.dma_start(out=b1t[:], in_=d['b1'].ap())

                if si == 0:
                    # HAM warmup: keep the PE busy during the initial
                    # x/w DMA window so the clock gate opens (K=8/8)
                    # before the first real matmul.  b1t arrives in <1us;
                    # ~96 small fp32 MMs cover the ~3.4us activity window
                    # and the remaining DMA latency at zero cost.
                    ps_w = ps1_pool.tile([HT, HT], f32, tag="warm")
                    for _ in range(96):
                        nc.tensor.matmul(ps_w[:], b1t[:], b1t[:],
                                         start=True, stop=True)

                if h8n:
                    h8_sb = h_pool.tile([128, h8n, sz], f8, tag="h8", name="h8")
                if h16n:
                    h16_sb = h_pool.tile([128, h16n * sz], f16, tag="h16",
                                         name="h16")

                xt8 = (xt_pool.tile([128, k8, sz], f8, tag="xt8", name="xt8")
                       if k8 else None)
                xt16 = (xt_pool.tile([128, k16 * sz], f16, tag="xt16",
                                     name="xt16") if k16 else None)

                for h in range(HT):
                    if k8:
                        w18t = w1_pool.tile([128, k8, 128], f8, tag="w18")
                        nc.sync.dma_start(out=w18t[:], in_=d['w18'].ap()[h])
                    if k16:
                        w116t = w1_pool.tile([128, k16 * 128], f16,
                                             tag="w116")
                        nc.sync.dma_start(out=w116t[:],
                                          in_=d['w116'].ap()[h])
                    ps = ps1_pool.tile([128, sz], f32, tag="ps1")
                    nk = kf1 + k16  # total MM instructions this h-tile
                    mi = 0
                    for p in range(kf1):
                        if h == 0 and p % 5 == 0:
                            ksl = slice(2 * p, min(2 * p + 10, k8))
                            nc.sync.dma_start(out=xt8[:, ksl, :],
                                              in_=d['xt8'].ap()[:, ksl, :])
                        nc.tensor.matmul(ps[:], w18t[:, 2 * p:2 * p + 2, :],
                                         xt8[:, 2 * p:2 * p + 2, :],
                                         start=(mi == 0),
                                         stop=(mi == nk - 1),
                                         perf_mode=DR)
                        mi += 1
                    for k in range(k16):
                        if h == 0 and k % 10 == 0:
                            cols = slice(k * sz, min((k + 10) * sz,
                                                     k16 * sz))
                            nc.sync.dma_start(out=xt16[:, cols],
                                              in_=d['xt16'].ap()[:, cols])
                        nc.tensor.matmul(ps[:], w116t[:, ts(k, 128)],
                                         xt16[:, ts(k, sz)],
                                         start=(mi == 0),
                                         stop=(mi == nk - 1))
                        mi += 1
                    if h < h8n:
                        nc.scalar.activation(h8_sb[:, h, :], ps[:], relu,
                                             bias=b1t[:, h:h + 1],
                                             scale=1.0 / W_SCALE)
                    else:
                        nc.scalar.activation(
                            h16_sb[:, ts(h - h8n, sz)], ps[:], relu,
                            bias=b1t[:, h:h + 1], scale=1.0 / W_SCALE)

                for o in range(OT):
                    if h8n:
                        w28t = w2_pool.tile([128, h8n, 128], f8, tag="w28")
                        nc.sync.dma_start(out=w28t[:], in_=d['w28'].ap()[o])
                    if h16n:
                        w216t = w2_pool.tile([128, h16n * 128], f16,
                                             tag="w216")
                        nc.sync.dma_start(out=w216t[:],
                                          in_=d['w216'].ap()[o])
                    ps2 = ps2_pool.tile([128, sz], f32, tag="ps2")
                    nk = kf2 + h16n
                    mi = 0
                    for p in range(kf2):
                        nc.tensor.matmul(ps2[:], w28t[:, 2 * p:2 * p + 2, :],
                                         h8_sb[:, 2 * p:2 * p + 2, :],
                                         start=(mi == 0),
                                         stop=(mi == nk - 1),
                                         perf_mode=DR)
                        mi += 1
                    for k in range(h16n):
                        nc.tensor.matmul(ps2[:], w216t[:, ts(k, 128)],
                                         h16_sb[:, ts(k, sz)],
                                         start=(mi == 0),
                                         stop=(mi == nk - 1))
                        mi += 1
                    yt_sb = y_pool.tile([128, sz], f16, tag="y")
                    nc.vector.tensor_copy(yt_sb[:], ps2[:])
                    nc.sync.dma_start(out=d['yt'].ap()[o], in_=yt_sb[:])

    nc.compile()
    return nc


def _get_nc(slot_key):
    if slot_key not in _NC:
        _NC[slot_key] = _build_bass(slot_key)
    return _NC[slot_key]


def _pack_x16(xb):
    """xb [s, k16*128] fp32 -> [128, k16*s] fp16 (k-tile major)."""
    s = xb.shape[0]
    kt = xb.shape[1] // 128
    return np.ascontiguousarray(
        xb.T.reshape(kt, 128, s).transpose(1, 0, 2)
    ).reshape(128, kt * s).astype(np.float16)


def _pack_x8(xb):
    """xb [s, k8*128] fp32 -> [128, k8, s] fp8."""
    s = xb.shape[0]
    kt = xb.shape[1] // 128
    return np.ascontiguousarray(
        xb.T.reshape(kt, 128, s).transpose(1, 0, 2)
    ).astype(ml_dtypes.float8_e4m3)


def kernel(id_emb, llm_emb, W1, b1, W2, b2, gW1, gb1, gW2, gb2):
    global LAST_RESULT
    from concourse.bass_utils import run_bass_kernel_spmd

    X = np.concatenate([np.asarray(id_emb, np.float32),
                        np.asarray(llm_emb, np.float32)], axis=1)
    W1 = np.asarray(W1, np.float32); b1 = np.asarray(b1, np.float32)
    W2 = np.asarray(W2, np.float32); b2 = np.asarray(b2, np.float32)

    top2, wts = _route(X, np.asarray(gW1), np.asarray(gb1),
                       np.asarray(gW2), np.asarray(gb2))
    gstar = _waterfill(wts)                     # [B, 2]

    # per-expert pair lists sorted by g* DESC (most fp8-eligible first)
    ids_e, w_e, g_e, counts = [], [], [], []
    for e in range(E):
        mask = (top2 == e)
        rows = np.nonzero(mask.any(axis=1))[0]
        w = wts[mask]
        gs = gstar[mask]
        o = np.argsort(-gs, kind="stable")
        ids_e.append(rows[o]); w_e.append(w[o]); g_e.append(gs[o])
        counts.append(len(rows))

    # eligibility counts per level
    elig_cnt = []
    for e in range(E):
        gs = g_e[e]
        elig_cnt.append([int((gs >= g - 1e-9).sum()) for g in LVL_G])

    slots, n = _solve(counts, elig_cnt)
    # device program order: smallest slot first (fastest first-DMA ->
    # earlier first matmul), rest descending so the tail slot is small too
    prog = sorted(range(len(slots)), key=lambda i: slots[i][1])
    prog = prog[:1] + prog[1:][::-1]
    slots = [slots[i] for i in prog]
    n = [[r[i] for i in prog] for r in n]
    slot_key = tuple(slots)

    # placement: slots desc g, per expert pointer into desc-g* list
    order = _slot_order(slots)
    blocks = {}
    ptr = [0] * E
    for i in order:
        lvl, sz = slots[i]
        g_lvl = LVL_G[lvl]
        copy = 0
        for e in range(E):
            for _ in range(n[e][i]):
                avail = elig_cnt[e][lvl] - ptr[e]
                take = max(min(sz, avail), 0)
                sel = slice(ptr[e], ptr[e] + take)
                ptr[e] += take
                blocks[(i, copy)] = (e, ids_e[e][sel], w_e[e][sel])
                copy += 1
        while copy < NCORES:
            blocks[(i, copy)] = (0, np.empty(0, np.int64),
                                 np.empty(0, np.float32))
            copy += 1
    for e in range(E):
        assert ptr[e] >= counts[e], (e, ptr[e], counts[e])

    # weight packing caches
    w1p8, w1p16, w2p8, w2p16, b1p = {}, {}, {}, {}, {}

    def get_w18(e, k8):
        key = (e, k8)
        if key not in w1p8:
            w1p8[key] = np.ascontiguousarray(
                (W1[e][:k8 * 128] * W_SCALE)
                .reshape(k8, 128, HT, 128).transpose(2, 1, 0, 3)
            ).astype(ml_dtypes.float8_e4m3)
        return w1p8[key]

    def get_w116(e, k8):
        key = (e, k8)
        if key not in w1p16:
            k16 = KT1 - k8
            w1p16[key] = np.ascontiguousarray(
                (W1[e][k8 * 128:] * W_SCALE)
                .reshape(k16, 128, HT, 128).transpose(2, 1, 0, 3)
            ).reshape(HT, 128, k16 * 128).astype(np.float16)
        return w1p16[key]

    def get_w28(e, h8):
        key = (e, h8)
        if key not in w2p8:
            w2p8[key] = np.ascontiguousarray(
                (W2[e][:h8 * 128] * W_SCALE)
                .reshape(h8, 128, OT, 128).transpose(2, 1, 0, 3)
            ).astype(ml_dtypes.float8_e4m3)
        return w2p8[key]

    def get_w216(e, h8):
        key = (e, h8)
        if key not in w2p16:
            h16 = KT2 - h8
            w2p16[key] = np.ascontiguousarray(
                (W2[e][h8 * 128:] * W_SCALE)
                .reshape(h16, 128, OT, 128).transpose(2, 1, 0, 3)
            ).reshape(OT, 128, h16 * 128).astype(np.float16)
        return w2p16[key]

    def get_b1(e):
        if e not in b1p:
            b1p[e] = np.ascontiguousarray(b1[e].reshape(HT, 128).T)
        return b1p[e]

    in_maps = [dict() for _ in range(NCORES)]
    for (i, copy), (e, ids, w) in blocks.items():
        lvl, sz = slots[i]
        kf1, kf2 = LEVELS[lvl]
        k8 = 2 * kf1
        k16 = KT1 - k8
        h8 = 2 * kf2
        h16 = KT2 - h8
        m = in_maps[copy]
        nt = len(ids)
        xb = np.zeros((sz, IN_DIM), np.float32)
        if nt:
            xb[:nt] = X[ids]
        if k8:
            m[f"xt8_{i}"] = _pack_x8(xb[:, :k8 * 128])
            m[f"w18_{i}"] = get_w18(e, k8)
        if k16:
            m[f"xt16_{i}"] = _pack_x16(xb[:, k8 * 128:])
            m[f"w116_{i}"] = get_w116(e, k8)
        if h8:
            m[f"w28_{i}"] = get_w28(e, h8)
        if h16:
            m[f"w216_{i}"] = get_w216(e, h8)
        m[f"b1_{i}"] = get_b1(e)

    nc = _get_nc(slot_key)
    trace = bool(int(os.environ.get("KERNEL_TRACE", "0")))
    res = run_bass_kernel_spmd(nc, in_maps, list(range(NCORES)), trace=trace)
    LAST_RESULT = res

    out = np.zeros((B, OUT_DIM), np.float32)
    for (i, copy), (e, ids, w) in blocks.items():
        if not len(ids):
            continue
        lvl, sz = slots[i]
        yt = np.asarray(res.results[copy][f"yt_{i}"]).astype(np.float32)
        y = yt.transpose(2, 0, 1).reshape(sz, OUT_DIM)[:len(ids)] / W_SCALE
        out[ids] += w[:, None] * (y + b2[e][None, :])
    return out


# revision 20
# speedup vs baseline: 1.0262x; 1.0262x over previous
"""MoE adapter kernel for 8 Trainium2 NeuronCores — v6.

v5 structure (slot-based SPMD, host routing/combine) with per-slot
mixed-precision K-splits: each slot has a level (KF1, KF2) = number of
fp8-DoubleRow k-tile pairs in the W1 / W2 contraction; the rest of the
contraction runs fp16.  Per-token error budget is allocated across the
token's two experts by a savings-per-error waterfill (g* per pair), so
top-1 experts also get partial fp8 instead of the v5 binary w<=0.48
rule.  Both W1 and W2 (fp8 and fp16 halves alike) are pre-scaled by 64
so mixed-dtype accumulation shares one PSUM scale; y is emitted fp16
(x64) and rescaled on host.

Cost model (ns, fitted from HW traces): fp16 MM = s/2.4+2.5 per k-tile;
fp8 DR pair = max(213.3, 0.4708*s); DMA = bytes/0.32GB/ms; 5us/slot.
"""

import os
import numpy as np
import ml_dtypes

B = 8192
IN_DIM = 5120
HID = 4096
OUT_DIM = 2048
E = 8
NCORES = 8
KT1 = IN_DIM // 128      # 40 k-tiles for W1
HT = HID // 128          # 32 h-tiles
KT2 = HID // 128         # 32 k-tiles for W2
OT = OUT_DIM // 128      # 16 o-tiles

W_SCALE = 64.0
C_BUDGET = 0.505         # per-element error cap (in w*sqrt(g) units)

# savings per unit f1 (W1 fp8 fraction) / f2 (W2 fp8 fraction), relative
# to the fp16 cost of one token-expert pair
_S1U = 0.311
_S2U = 0.124

# slot levels: (KF1 DR pairs of 20, KF2 DR pairs of 16) -> g = KF1/20+KF2/16
LEVELS = [(0, 0), (5, 0), (10, 0), (15, 0), (20, 0), (20, 8), (20, 16)]
LVL_G = [kf1 / 20.0 + kf2 / 16.0 for (kf1, kf2) in LEVELS]

LAST_RESULT = None

import random as _random

_MAXS = 512
_MINS = 320
_SLOT_PEN = 5000.0
_DMA_GBPNS = 320.0       # bytes per ns (~320 GB/s effective per core)


def _mm16(s):
    return s / 2.4 + 2.5


def _mm8(s):
    return max(213.3, 0.4708 * s)


def _cost_slot(lvl, s):
    kf1, kf2 = LEVELS[lvl]
    comp1 = HT * (kf1 * _mm8(s) + (KT1 - 2 * kf1) * _mm16(s))
    comp2 = OT * (kf2 * _mm8(s) + (KT2 - 2 * kf2) * _mm16(s))
    w1by = 128 * 128 * (2 * kf1 + (KT1 - 2 * kf1) * 2) * HT
    w2by = 128 * 128 * (2 * kf2 + (KT2 - 2 * kf2) * 2) * OT
    xby = 128 * s * (2 * kf1 + (KT1 - 2 * kf1) * 2)
    yby = 128 * s * 2 * OT
    # per-phase roofline: W1 phase streams w1+x, W2 phase streams w2+y
    ph1 = max(comp1, (w1by + xby) / _DMA_GBPNS)
    ph2 = max(comp2, (w2by + yby) / _DMA_GBPNS)
    return ph1 + ph2 + _SLOT_PEN


def _slot_order(slots):
    """Placement order: descending g (strictest eligibility first)."""
    return sorted(range(len(slots)), key=lambda i: (-LVL_G[slots[i][0]],
                                                    -slots[i][1]))


def _coverage_all(slots, n, elig_cnt):
    """Greedy nested coverage.  elig_cnt[e][lvl] = #pairs of e with
    g* >= LVL_G[lvl].  Returns per-expert covered counts."""
    order = _slot_order(slots)
    covered = [0] * E
    for e in range(E):
        taken = 0
        for i in order:
            lvl, s = slots[i]
            cap = n[e][i] * s
            avail = elig_cnt[e][lvl] - taken
            if avail > 0 and cap > 0:
                taken += min(cap, avail)
        covered[e] = taken
    return covered


def _makespan(slots, n, counts, elig_cnt):
    cost = sum(_cost_slot(lvl, s) for (lvl, s) in slots)
    pen = 0.0
    covered = _coverage_all(slots, n, elig_cnt)
    for e in range(E):
        deficit = counts[e] - covered[e]
        if deficit > 0:
            pen += 50000.0 + deficit * 7000.0
    for i in range(len(slots)):
        used = sum(n[e][i] for e in range(E))
        if used > NCORES:
            pen += (used - NCORES) * 3e6
    return cost + pen


def _greedy0(counts):
    """All-fp16 fallback construction (level 0 slots of size 448)."""
    slots = []
    n = [[] for _ in range(E)]
    need = list(counts)
    while any(x > 0 for x in need):
        slots.append((0, 448))
        for e in range(E):
            n[e].append(0)
        copies = NCORES
        for e in sorted(range(E), key=lambda e: -need[e]):
            while need[e] > 0 and copies > 0:
                n[e][-1] += 1
                need[e] -= 448
                copies -= 1
    return slots, n


def _seed_solution(counts, elig_cnt):
    """Heuristic start: per expert, walk pairs desc g*, make 512 slots at
    the floored level of each chunk's min g*."""
    slots = []
    n = [[] for _ in range(E)]
    # aggregate: for each level desc, capacity needed
    for lvl in range(len(LEVELS) - 1, -1, -1):
        need = []
        for e in range(E):
            hi = elig_cnt[e][lvl]
            lo = elig_cnt[e][lvl + 1] if lvl + 1 < len(LEVELS) else 0
            need.append(hi - lo)
        while any(x > 0 for x in need):
            slots.append((lvl, 512))
            for e in range(E):
                n[e].append(0)
            copies = NCORES
            for e in sorted(range(E), key=lambda e: -need[e]):
                while need[e] > 0 and copies > 0:
                    n[e][-1] += 1
                    need[e] -= 512
                    copies -= 1
    return slots, n


def _seed_v5ish(counts, elig_cnt):
    """v5-style: level-4 (g=1) slots covering eligible pairs, rest fp16."""
    slots = []
    n = [[] for _ in range(E)]
    lvl4 = LVL_G.index(1.0)
    need = [elig_cnt[e][lvl4] for e in range(E)]
    while sum(need) > 2048:
        slots.append((lvl4, 464))
        for e in range(E):
            n[e].append(0)
        copies = NCORES
        for e in sorted(range(E), key=lambda e: -need[e]):
            while need[e] > 0 and copies > 0:
                n[e][-1] += 1
                need[e] -= 464
                copies -= 1
    placed = [min(elig_cnt[e][lvl4] - max(need[e], 0), elig_cnt[e][lvl4])
              for e in range(E)]
    need = [counts[e] - placed[e] for e in range(E)]
    while any(x > 0 for x in need):
        slots.append((0, 448))
        for e in range(E):
            n[e].append(0)
        copies = NCORES
        for e in sorted(range(E), key=lambda e: -need[e]):
            while need[e] > 0 and copies > 0:
                n[e][-1] += 1
                need[e] -= 448
                copies -= 1
    return slots, n


def _solve_once(counts, elig_cnt, iters, seed, init=None):
    rng = _random.Random(seed)
    if init is not None:
        slots = [s for s in init[0]]
        n = [r[:] for r in init[1]]
    else:
        slots, n = _seed_solution(counts, elig_cnt)
    cur = _makespan(slots, n, counts, elig_cnt)
    best = (cur, [s for s in slots], [r[:] for r in n])
    nlvl = len(LEVELS)
    for it in range(iters):
        T = max(0.02, 1.0 * (1 - it / iters))
        op = rng.random()
        slots2 = [s for s in slots]
        n2 = [r[:] for r in n]
        if op < 0.25 and slots2:
            i = rng.randrange(len(slots2))
            lvl, s = slots2[i]
            s2 = min(_MAXS, max(_MINS,
                                s + rng.choice([-64, -32, -16, 16, 32, 64])))
            slots2[i] = (lvl, s2)
        elif op < 0.45 and slots2:
            i = rng.randrange(len(slots2))
            lvl, s = slots2[i]
            lvl2 = min(nlvl - 1, max(0, lvl + rng.choice([-2, -1, 1, 2])))
            slots2[i] = (lvl2, s)
        elif op < 0.60 and slots2:
            i = rng.randrange(len(slots2))
            e1 = rng.randrange(E)
            if n2[e1][i] > 0:
                n2[e1][i] -= 1
                if rng.random() < 0.8:
                    n2[rng.randrange(E)][i] += 1
        elif op < 0.72 and slots2:
            n2[rng.randrange(E)][rng.randrange(len(slots2))] += 1
        elif op < 0.85 and len(slots2) > 1:
            i = rng.randrange(len(slots2))
            del slots2[i]
            for e in range(E):
                del n2[e][i]
        else:
            lvl = rng.randrange(nlvl)
            slots2.append((lvl, rng.choice(range(_MINS, _MAXS + 1, 16))))
            for e in range(E):
                n2[e].append(0)
            i = len(slots2) - 1
            copies = NCORES
            for e in sorted(range(E), key=lambda e: -counts[e])[:NCORES]:
                if copies:
                    n2[e][i] += 1
                    copies -= 1
        c2 = _makespan(slots2, n2, counts, elig_cnt)
        if c2 <= cur or rng.random() < pow(2.718, -(c2 - cur) / (T * 30000)):
            slots, n, cur = slots2, n2, c2
            if cur < best[0]:
                best = (cur, [s for s in slots], [r[:] for r in n])
    return best


def _used_lengths(slots, n, elig_cnt):
    """Mirror placement: per slot, max copy fill (for size shrinking)."""
    order = _slot_order(slots)
    used = [0] * len(slots)
    ptr = [0] * E
    for i in order:
        lvl, sz = slots[i]
        for e in range(E):
            for _ in range(n[e][i]):
                avail = elig_cnt[e][lvl] - ptr[e]
                take = max(min(sz, avail), 0)
                ptr[e] += take
                used[i] = max(used[i], take)
    return used


def _shrink(slots, n, counts, elig_cnt):
    """Shrink slot sizes to the max actually-used copy length."""
    for _ in range(4):
        used = _used_lengths(slots, n, elig_cnt)
        new = []
        changed = False
        for i, (lvl, sz) in enumerate(slots):
            s2 = min(sz, max(_MINS, -(-used[i] // 16) * 16))
            if s2 != sz:
                changed = True
            new.append((lvl, s2))
        cov = _coverage_all(new, n, elig_cnt)
        if all(cov[e] >= counts[e] for e in range(E)):
            slots = new
        else:
            break
        if not changed:
            break
    return slots


def _solve(counts, elig_cnt, restarts=10, iters=200000):
    best = None
    seeds = [None] * restarts
    seeds[0] = _seed_v5ish(counts, elig_cnt)
    if restarts > 1:
        seeds[1] = _seed_v5ish(counts, elig_cnt)
    for r in range(restarts):
        c, slots, n = _solve_once(counts, elig_cnt, iters, seed=r,
                                  init=seeds[r])
        slots = _shrink(slots, n, counts, elig_cnt)
        c = _makespan(slots, n, counts, elig_cnt)
        if best is None or c < best[0]:
            best = (c, slots, n)
    c, slots, n = best
    keep = [i for i in range(len(slots)) if any(n[e][i] for e in range(E))]
    slots = [slots[i] for i in keep]
    n = [[r[i] for i in keep] for r in n]

    # deterministic deficit repair: lower levels / grow / add fp16 slots
    for _ in range(128):
        covered = _coverage_all(slots, n, elig_cnt)
        bad = [e for e in range(E) if covered[e] < counts[e]]
        if not bad:
            break
        e = bad[0]
        fixed = False
        # 1. grow a slot this expert uses
        for i in range(len(slots)):
            lvl, s = slots[i]
            if n[e][i] and s + 16 <= _MAXS:
                slots[i] = (lvl, s + 16)
                fixed = True
                break
        if not fixed:
            # 2. lower the level of a slot this expert uses
            for i in range(len(slots)):
                lvl, s = slots[i]
                if n[e][i] and lvl > 0:
                    slots[i] = (lvl - 1, s)
                    fixed = True
                    break
        if not fixed:
            # 3. add a copy in an existing non-full slot (level 0 pref)
            for i in sorted(range(len(slots)),
                            key=lambda i: LVL_G[slots[i][0]]):
                if sum(n[x][i] for x in range(E)) < NCORES:
                    n[e][i] += 1
                    fixed = True
                    break
        if not fixed:
            slots.append((0, 448))
            for x in range(E):
                n[x].append(1 if x == e else 0)
    covered = _coverage_all(slots, n, elig_cnt)
    if any(covered[e] < counts[e] for e in range(E)):
        slots, n = _greedy0(counts)
    slots = _shrink(slots, n, counts, elig_cnt)
    return slots, n


def _route(X, gW1, gb1, gW2, gb2):
    g = np.maximum(X.astype(np.float64) @ gW1.astype(np.float64)
                   + gb1.astype(np.float64), 0.0)
    logits = g @ gW2.astype(np.float64) + gb2.astype(np.float64)
    top2 = np.argpartition(-logits, 1, axis=1)[:, :2]
    l2 = np.take_along_axis(logits, top2, axis=1)
    ew = np.exp(l2 - l2.max(axis=1, keepdims=True))
    wts = ew / ew.sum(axis=1, keepdims=True)
    return top2, wts.astype(np.float32)


def _waterfill(wts):
    """Per-token optimal error-budget split.  wts [B,2] combine weights.
    Returns g* [B,2]: total fp8 fraction units (f1+f2, in [0,2]) each
    pair can take s.t. sum_i w_i^2 g_i <= C^2, maximizing DR savings."""
    w = wts.astype(np.float64)
    n = w.shape[0]
    budget = np.full(n, C_BUDGET * C_BUDGET)
    # items per token: (expert0 W1), (e0 W2), (e1 W1), (e1 W2)
    costs = np.stack([w[:, 0] ** 2, w[:, 0] ** 2,
                      w[:, 1] ** 2, w[:, 1] ** 2], axis=1)
    sav = np.array([_S1U, _S2U, _S1U, _S2U])
    ratio = sav[None, :] / costs
    order = np.argsort(-ratio, axis=1)
    f = np.zeros((n, 4))
    for j in range(4):
        idx = order[:, j]
        c = np.take_along_axis(costs, idx[:, None], axis=1)[:, 0]
        take = np.minimum(1.0, budget / c)
        np.put_along_axis(f, idx[:, None], take[:, None], axis=1)
        budget -= take * c
    return np.stack([f[:, 0] + f[:, 1], f[:, 2] + f[:, 3]], axis=1)


_NC = {}


def _build_bass(slot_key):
    import concourse.mybir as mybir
    import concourse.tile as tile
    from concourse import bacc
    from concourse.bass import ts

    f16 = mybir.dt.float16
    f32 = mybir.dt.float32
    f8 = mybir.dt.float8e4
    DR = mybir.MatmulPerfMode.DoubleRow
    relu = mybir.ActivationFunctionType.Relu

    nc = bacc.Bacc("TRN2", target_bir_lowering=False, debug=False,
                   num_devices=NCORES)

    slots = list(slot_key)
    dram = []
    for si, (lvl, sz) in enumerate(slots):
        kf1, kf2 = LEVELS[lvl]
        k8 = 2 * kf1              # fp8 k-tiles in W1
        k16 = KT1 - k8            # fp16 k-tiles in W1
        h8 = 2 * kf2              # fp8 k-tiles (h-tiles) in W2
        h16 = KT2 - h8
        d = {}
        if k8:
            d['xt8'] = nc.dram_tensor(f"xt8_{si}", [128, k8, sz], f8,
                                      kind="ExternalInput")
            d['w18'] = nc.dram_tensor(f"w18_{si}", [HT, 128, k8, 128], f8,
                                      kind="ExternalInput")
        if k16:
            d['xt16'] = nc.dram_tensor(f"xt16_{si}", [128, k16 * sz], f16,
                                       kind="ExternalInput")
            d['w116'] = nc.dram_tensor(f"w116_{si}", [HT, 128, k16 * 128],
                                       f16, kind="ExternalInput")
        if h8:
            d['w28'] = nc.dram_tensor(f"w28_{si}", [OT, 128, h8, 128], f8,
                                      kind="ExternalInput")
        if h16:
            d['w216'] = nc.dram_tensor(f"w216_{si}", [OT, 128, h16 * 128],
                                       f16, kind="ExternalInput")
        d['b1'] = nc.dram_tensor(f"b1_{si}", [128, HT], f32,
                                 kind="ExternalInput")
        d['yt'] = nc.dram_tensor(f"yt_{si}", [OT, 128, sz], f16,
                                 kind="ExternalOutput")
        dram.append(d)

    with tile.TileContext(nc) as tc:
        with (
            tc.tile_pool(name="xt", bufs=2) as xt_pool,
            tc.tile_pool(name="w1", bufs=3) as w1_pool,
            tc.tile_pool(name="w2", bufs=2) as w2_pool,
            tc.tile_pool(name="h", bufs=1) as h_pool,
            tc.tile_pool(name="b", bufs=2) as b_pool,
            tc.tile_pool(name="y", bufs=4) as y_pool,
            tc.tile_pool(name="ps1", bufs=2, space="PSUM") as ps1_pool,
            tc.tile_pool(name="ps2", bufs=2, space="PSUM") as ps2_pool,
        ):
            for si, (lvl, sz) in enumerate(slots):
                kf1, kf2 = LEVELS[lvl]
                k8 = 2 * kf1
                k16 = KT1 - k8
                h8n = 2 * kf2
                h16n = KT2 - h8n
                d = dram[si]

                b1t = b_pool.tile([128, HT], f32, tag="b1")
                nc.sync.dma_start(out=b1t[:], in_=d['b1'].ap())

                if h8n:
                    h8_sb = h_pool.tile([128, h8n, sz], f8, tag="h8", name="h8")
                if h16n:
                    h16_sb = h_pool.tile([128, h16n * sz], f16, tag="h16",
                                         name="h16")

                xt8 = (xt_pool.tile([128, k8, sz], f8, tag="xt8", name="xt8")
                       if k8 else None)
                xt16 = (xt_pool.tile([128, k16 * sz], f16, tag="xt16",
                                     name="xt16") if k16 else None)

                for h in range(HT):
                    if k8:
                        w18t = w1_pool.tile([128, k8, 128], f8, tag="w18")
                        nc.sync.dma_start(out=w18t[:], in_=d['w18'].ap()[h])
                    if k16:
                        w116t = w1_pool.tile([128, k16 * 128], f16,
                                             tag="w116")
                        nc.sync.dma_start(out=w116t[:],
                                          in_=d['w116'].ap()[h])
                    ps = ps1_pool.tile([128, sz], f32, tag="ps1")
                    nk = kf1 + k16  # total MM instructions this h-tile
                    mi = 0
                    for p in range(kf1):
                        if h == 0 and p % 5 == 0:
                            ksl = slice(2 * p, min(2 * p + 10, k8))
                            nc.sync.dma_start(out=xt8[:, ksl, :],
                                              in_=d['xt8'].ap()[:, ksl, :])
                        nc.tensor.matmul(ps[:], w18t[:, 2 * p:2 * p + 2, :],
                                         xt8[:, 2 * p:2 * p + 2, :],
                                         start=(mi == 0),
                                         stop=(mi == nk - 1),
                                         perf_mode=DR)
                        mi += 1
                    for k in range(k16):
                        if h == 0 and k % 10 == 0:
                            cols = slice(k * sz, min((k + 10) * sz,
                                                     k16 * sz))
                            nc.sync.dma_start(out=xt16[:, cols],
                                              in_=d['xt16'].ap()[:, cols])
                        nc.tensor.matmul(ps[:], w116t[:, ts(k, 128)],
                                         xt16[:, ts(k, sz)],
                                         start=(mi == 0),
                                         stop=(mi == nk - 1))
                        mi += 1
                    if h < h8n:
                        nc.scalar.activation(h8_sb[:, h, :], ps[:], relu,
                                             bias=b1t[:, h:h + 1],
                                             scale=1.0 / W_SCALE)
                    else:
                        nc.scalar.activation(
                            h16_sb[:, ts(h - h8n, sz)], ps[:], relu,
                            bias=b1t[:, h:h + 1], scale=1.0 / W_SCALE)

                for o in range(OT):
                    if h8n:
                        w28t = w2_pool.tile([128, h8n, 128], f8, tag="w28")
                        nc.sync.dma_start(out=w28t[:], in_=d['w28'].ap()[o])
                    if h16n:
                        w216t = w2_pool.tile([128, h16n * 128], f16,
                                             tag="w216")
                        nc.sync.dma_start(out=w216t[:],
                                          in_=d['w216'].ap()[o])
                    ps2 = ps2_pool.tile([128, sz], f32, tag="ps2")
                    nk = kf2 + h16n
                    mi = 0
                    for p in range(kf2):
                        nc.tensor.matmul(ps2[:], w28t[:, 2 * p:2 * p + 2, :],
                                         h8_sb[:, 2 * p:2 * p + 2, :],
                                         start=(mi == 0),
                                         stop=(mi == nk - 1),
                                         perf_mode=DR)
                        mi += 1
                    for k in range(h16n):
                        nc.tensor.matmul(ps2[:], w216t[:, ts(k, 128)],
                                         h16_sb[:, ts(k, sz)],
                                         start=(mi == 0),
                                         stop=(mi == nk - 1))
                        mi += 1
                    yt_sb = y_pool.tile([128, sz], f16, tag="y")
                    nc.vector.tensor_copy(yt_sb[:], ps2[:])
                    nc.sync.dma_start(out=d['yt'].ap()[o], in_=yt_sb[:])

    nc.compile()
    return nc


def _get_nc(slot_key):
    if slot_key not in _NC:
        _NC[slot_key] = _build_bass(slot_key)
    return _NC[slot_key]


def _pack_x16(xb):
    """xb [s, k16*128] fp32 -> [128, k16*s] fp16 (k-tile major)."""
    s = xb.shape[0]
    kt = xb.shape[1] // 128
    return np.ascontiguousarray(
        xb.T.reshape(kt, 128, s).transpose(1, 0, 2)
    ).reshape(128, kt * s).astype(np.float16)


def _pack_x8(xb):
    """xb [s, k8*128] fp32 -> [128, k8, s] fp8."""
    s = xb.shape[0]
    kt = xb.shape[1] // 128
    return np.ascontiguousarray(
        xb.T.reshape(kt, 128, s).transpose(1, 0, 2)
    ).astype(ml_dtypes.float8_e4m3)


def kernel(id_emb, llm_emb, W1, b1, W2, b2, gW1, gb1, gW2, gb2):
    global LAST_RESULT
    from concourse.bass_utils import run_bass_kernel_spmd

    X = np.concatenate([np.asarray(id_emb, np.float32),
                        np.asarray(llm_emb, np.float32)], axis=1)
    W1 = np.asarray(W1, np.float32); b1 = np.asarray(b1, np.float32)
    W2 = np.asarray(W2, np.float32); b2 = np.asarray(b2, np.float32)

    top2, wts = _route(X, np.asarray(gW1), np.asarray(gb1),
                       np.asarray(gW2), np.asarray(gb2))
    gstar = _waterfill(wts)                     # [B, 2]

    # per-expert pair lists sorted by g* DESC (most fp8-eligible first)
    ids_e, w_e, g_e, counts = [], [], [], []
    for e in range(E):
        mask = (top2 == e)
        rows = np.nonzero(mask.any(axis=1))[0]
        w = wts[mask]
        gs = gstar[mask]
        o = np.argsort(-gs, kind="stable")
        ids_e.append(rows[o]); w_e.append(w[o]); g_e.append(gs[o])
        counts.append(len(rows))

    # eligibility counts per level
    elig_cnt = []
    for e in range(E):
        gs = g_e[e]
        elig_cnt.append([int((gs >= g - 1e-9).sum()) for g in LVL_G])

    slots, n = _solve(counts, elig_cnt)
    # device program order: smallest slot first (fastest first-DMA ->
    # earlier first matmul), rest descending so the tail slot is small too
    prog = sorted(range(len(slots)), key=lambda i: slots[i][1])
    prog = prog[:1] + prog[1:][::-1]
    slots = [slots[i] for i in prog]
    n = [[r[i] for i in prog] for r in n]
    slot_key = tuple(slots)

    # placement: slots desc g, per expert pointer into desc-g* list
    order = _slot_order(slots)
    blocks = {}
    ptr = [0] * E
    for i in order:
        lvl, sz = slots[i]
        g_lvl = LVL_G[lvl]
        copy = 0
        for e in range(E):
            for _ in range(n[e][i]):
                avail = elig_cnt[e][lvl] - ptr[e]
                take = max(min(sz, avail), 0)
                sel = slice(ptr[e], ptr[e] + take)
                ptr[e] += take
                blocks[(i, copy)] = (e, ids_e[e][sel], w_e[e][sel])
                copy += 1
        while copy < NCORES:
            blocks[(i, copy)] = (0, np.empty(0, np.int64),
                                 np.empty(0, np.float32))
            copy += 1
    for e in range(E):
        assert ptr[e] >= counts[e], (e, ptr[e], counts[e])

    # weight packing caches
    w1p8, w1p16, w2p8, w2p16, b1p = {}, {}, {}, {}, {}

    def get_w18(e, k8):
        key = (e, k8)
        if key not in w1p8:
            w1p8[key] = np.ascontiguousarray(
                (W1[e][:k8 * 128] * W_SCALE)
                .reshape(k8, 128, HT, 128).transpose(2, 1, 0, 3)
            ).astype(ml_dtypes.float8_e4m3)
        return w1p8[key]

    def get_w116(e, k8):
        key = (e, k8)
        if key not in w1p16:
            k16 = KT1 - k8
            w1p16[key] = np.ascontiguousarray(
                (W1[e][k8 * 128:] * W_SCALE)
                .reshape(k16, 128, HT, 128).transpose(2, 1, 0, 3)
            ).reshape(HT, 128, k16 * 128).astype(np.float16)
        return w1p16[key]

    def get_w28(e, h8):
        key = (e, h8)
        if key not in w2p8:
            w2p8[key] = np.ascontiguousarray(
                (W2[e][:h8 * 128] * W_SCALE)
                .reshape(h8, 128, OT, 128).transpose(2, 1, 0, 3)
            ).astype(ml_dtypes.float8_e4m3)
        return w2p8[key]

    def get_w216(e, h8):
        key = (e, h8)
        if key not in w2p16:
            h16 = KT2 - h8
            w2p16[key] = np.ascontiguousarray(
                (W2[e][h8 * 128:] * W_SCALE)
                .reshape(h16, 128, OT, 128).transpose(2, 1, 0, 3)
            ).reshape(OT, 128, h16 * 128).astype(np.float16)
        return w2p16[key]

    def get_b1(e):
        if e not in b1p:
            b1p[e] = np.ascontiguousarray(b1[e].reshape(HT, 128).T)
        return b1p[e]

    in_maps = [dict() for _ in range(NCORES)]
    for (i, copy), (e, ids, w) in blocks.items():
        lvl, sz = slots[i]
        kf1, kf2 = LEVELS[lvl]
        k8 = 2 * kf1
        k16 = KT1 - k8
        h8 = 2 * kf2
        h16 = KT2 - h8
        m = in_maps[copy]
        nt = len(ids)
        xb = np.zeros((sz, IN_DIM), np.float32)
        if nt:
            xb[:nt] = X[ids]
        if k8:
            m[f"xt8_{i}"] = _pack_x8(xb[:, :k8 * 128])
            m[f"w18_{i}"] = get_w18(e, k8)
        if k16:
            m[f"xt16_{i}"] = _pack_x16(xb[:, k8 * 128:])
            m[f"w116_{i}"] = get_w116(e, k8)
        if h8:
            m[f"w28_{i}"] = get_w28(e, h8)
        if h16:
            m[f"w216_{i}"] = get_w216(e, h8)
        m[f"b1_{i}"] = get_b1(e)

    nc = _get_nc(slot_key)
    trace = bool(int(os.environ.get("KERNEL_TRACE", "0")))
    res = run_bass_kernel_spmd(nc, in_maps, list(range(NCORES)), trace=trace)
    LAST_RESULT = res

    out = np.zeros((B, OUT_DIM), np.float32)
    for (i, copy), (e, ids, w) in blocks.items():
        if not len(ids):
            continue
        lvl, sz = slots[i]
        yt = np.asarray(res.results[copy][f"yt_{i}"]).astype(np.float32)
        y = yt.transpose(2, 0, 1).reshape(sz, OUT_DIM)[:len(ids)] / W_SCALE
        out[ids] += w[:, None] * (y + b2[e][None, :])
    return out
